# revision 1
# baseline (speedup 1.0000x reference)
"""DCTHFClip kernel for 8 Trainium2 NeuronCores.

Math: the reference computes
    x_dct   = C @ x          (DCT-II along S, per (batch, feature) column)
    m       = |mean_{b,d} x_dct|          (shape (S,))
    thr     = quantile(m, 0.7); last_index = last k with m[k] > thr
    trunc   = x_dct[:, :last_index, :]                  (fp32 output)
    recon   = Cl^T @ trunc  with Cl = dct_matrix(L)     (fp16 output)

Key reductions used here:
  1. m is LINEAR in x:  m = |C @ mean_{b,d}(x)| — a length-S vector DCT,
     computed on host for pennies.  That resolves L before any device work
     (mirrors the host-sync .item() in the original torch code).
  2. recon = (Cl^T @ C[:L]) @ x = G @ x — so both outputs are plain
     matmuls of the SAME input x against host-precomputed matrices.
     Stack W = [C[:L]; G] (2L x S): one stationary-weight matmul per core
     computes both outputs, batch-sharded across the 8 cores, with no
     cross-core communication.
Matmuls run in fp16 x fp16 (fp32 PSUM accumulate): full PE rate with the
LDWEIGHTS hidden behind the previous matmul; input/weight quantization
contributes ~2e-4 relative error, well inside comparison tolerances.
"""

import os
import sys

import numpy as np

_B, _S, _D = 64, 576, 1024
_NCORES = 8
_P = 128          # SBUF partitions
_NT = 512         # matmul moving free dim (one fp32 PSUM bank)

_CACHE = {}
LAST_RESULTS = None  # stashed BassKernelResults for test.py profiling


def _ensure_paths():
    for p in ("/root/.axon_site", "/root/.axon_site/_ro/trn_rl_repo",
              "/root/.axon_site/_ro/pypackages", "/opt/trn_rl_repo", "/opt/pypackages"):
        if os.path.isdir(p) and p not in sys.path:
            sys.path.append(p)


def _dct_matrix64(n):
    k = np.arange(n)[:, None].astype(np.float64)
    i = np.arange(n)[None, :].astype(np.float64)
    C = np.cos(np.pi / n * (i + 0.5) * k)
    scale = np.where(k == 0, np.sqrt(1.0 / n), np.sqrt(2.0 / n))
    return C * scale  # (n_freq, n_pos)


def _resolve_L(x):
    """Host-side: trunc length via linearity of the batch/feature mean."""
    S = x.shape[1]
    xbar = x.mean(axis=(0, 2), dtype=np.float64)  # (S,)
    C = _dct_matrix64(S)
    m = np.abs(C @ xbar)
    thr = np.quantile(m, 0.7)
    idx = np.nonzero(m > thr)[0]
    last_index = int(idx[-1]) if idx.size > 0 else -1
    # mirror python slice semantics of x_dct[:, :last_index, :]
    return len(range(S)[:last_index])


def _build_weights(S, L):
    C = _dct_matrix64(S)          # (S, S)
    Ct = C[:L]                    # (L, S)
    Cl = _dct_matrix64(L)         # (L, L)
    G = Cl.T @ Ct                 # (L, S)  recon = G @ x
    W = np.concatenate([Ct, G], axis=0)          # (2L, S)
    # fp16 weights: 10-bit mantissa keeps |err| ~3e-4 on unit-RMS outputs
    # while enabling the separate LDWEIGHTS path (hidden by the PE reorder
    # window) instead of fp32r's serialized self-load.
    return np.ascontiguousarray(W.T.astype(np.float16))  # lhsT layout (S, 2L)


def _build_program(Bc, S, D, L):
    _ensure_paths()
    import concourse.bacc as bacc
    import concourse.mybir as mybir
    import concourse.tile as tile

    f32 = mybir.dt.float32
    f32r = mybir.dt.float32r
    f16 = mybir.dt.float16

    P, NT = _P, _NT
    M2 = 2 * L
    KT = (S + P - 1) // P        # contraction tiles (5)
    MT = (M2 + P - 1) // P       # output-row tiles (9)
    NG = D // NT                 # moving-dim groups per batch (2)
    kfull = S // P               # full contraction tiles (4)
    krem = S - kfull * P         # remainder rows (64)

    nc = bacc.Bacc("TRN2", target_bir_lowering=False, debug=False,
                   num_devices=_NCORES)
    x_d = nc.dram_tensor("x", [Bc, S, D], f16, kind="ExternalInput")
    wt_d = nc.dram_tensor("wt", [S, M2], f16, kind="ExternalInput")
    tr_d = nc.dram_tensor("trunc", [Bc, L, D], f32, kind="ExternalOutput")
    rc_d = nc.dram_tensor("recon", [Bc, L, D], f16, kind="ExternalOutput")

    with tile.TileContext(nc) as tc:
        with (
            tc.tile_pool(name="wpool", bufs=1) as wpool,
            tc.tile_pool(name="xpool", bufs=4) as xpool,
            tc.tile_pool(name="spool", bufs=6) as spool,
            tc.tile_pool(name="psum", bufs=8, space="PSUM") as psum_pool,
        ):
            # weights: the kt=4 remainder (64 rows) is loaded twice, at
            # partitions 0-63 and 64-127, so the two n-groups' remainder
            # matmuls can run concurrently as row-tiled pairs.
            # weight loads ride the scalar engine's HWDGE ring so they
            # stream in parallel with the x loads on the sync ring.
            wt_sb = wpool.tile([P, KT, M2], f16)
            for kt in range(kfull):
                nc.scalar.dma_start(out=wt_sb[:, kt, :],
                                    in_=wt_d[kt * P:(kt + 1) * P, :])
            if krem:
                nc.scalar.dma_start(out=wt_sb[0:krem, kfull, :],
                                    in_=wt_d[kfull * P:S, :])
                nc.scalar.dma_start(out=wt_sb[P - krem:P, kfull, :],
                                    in_=wt_d[kfull * P:S, :])

            for b in range(Bc):
                # full-width x rows -> 2KB DMA packets (1KB halves measurably
                # hurt HBM efficiency)
                x_sb = xpool.tile([P, KT, D], f16, tag="x")
                if b == 0:
                    # batch 0 split per k-tile: the first matmul only gates
                    # on the first 294KB instead of the whole 1.2MB
                    for kt in range(kfull):
                        nc.sync.dma_start(out=x_sb[:, kt, :],
                                          in_=x_d[b, kt * P:(kt + 1) * P, :])
                else:
                    nc.sync.dma_start(
                        out=x_sb[:, 0:kfull, :],
                        in_=x_d[b, 0:kfull * P, :].rearrange(
                            "(kt p) d -> p kt d", p=P))
                if krem:
                    nc.sync.dma_start(out=x_sb[0:krem, kfull, :],
                                      in_=x_d[b, kfull * P:S, :])
                    nc.sync.dma_start(out=x_sb[P - krem:P, kfull, NT:D],
                                      in_=x_d[b, kfull * P:S, NT:D])
                for mt in range(MT - 1, -1, -1):
                    r0 = mt * P
                    rows = min(P, M2 - r0)
                    t_rows = max(0, min(L - r0, rows))
                    # full-width staging (both n-groups) -> one big DMA out
                    st = (spool.tile([P, D], f32, tag="st", name="st")
                          if t_rows > 0 else None)
                    sr = (spool.tile([P, D], f16, tag="sr", name="sr")
                          if t_rows < rows else None)
                    pss = [psum_pool.tile([P, NT], f32, tag="ps", name="ps")
                           for _ in range(NG)]
                    # kt-major: both n-groups' matmuls fire as soon as one
                    # k-tile of x lands — keeps the PE fed while batch-0
                    # pieces stream in
                    for kt in range(kfull):
                        for ng in range(NG):
                            n0 = ng * NT
                            nc.tensor.matmul(
                                pss[ng][0:rows, :],
                                wt_sb[0:P, kt, r0:r0 + rows],
                                x_sb[0:P, kt, n0:n0 + NT],
                                start=(kt == 0),
                                stop=False,
                            )
                    # kt=4 remainder: the two n-groups' 64-row matmuls sit in
                    # disjoint PE row groups (partitions 0-63 / 64-127) and
                    # execute concurrently.
                    nc.tensor.matmul(
                        pss[0][0:rows, :],
                        wt_sb[0:krem, kfull, r0:r0 + rows],
                        x_sb[0:krem, kfull, 0:NT],
                        start=False, stop=True,
                    )
                    nc.tensor.matmul(
                        pss[1][0:rows, :],
                        wt_sb[P - krem:P, kfull, r0:r0 + rows],
                        x_sb[P - krem:P, kfull, NT:D],
                        start=False, stop=True,
                    )
                    for ng in range(NG):
                        n0 = ng * NT
                        ps = pss[ng]
                        if st is not None:
                            nc.vector.tensor_copy(st[0:t_rows, n0:n0 + NT],
                                                  ps[0:t_rows, :])
                        if sr is not None:
                            # fp32->fp16 cast on the scalar engine; the DVE
                            # cast intermittently emits top-nibble-quantized
                            # values under load (HW quirk). Engine APs from a
                            # non-zero partition base max out at 32
                            # partitions, so cast [0:rows] and DMA out only
                            # the recon rows.
                            nc.scalar.copy(sr[0:rows, n0:n0 + NT],
                                           ps[0:rows, :])
                    if st is not None:
                        nc.sync.dma_start(out=tr_d[b, r0:r0 + t_rows, :],
                                          in_=st[0:t_rows, :])
                    if sr is not None:
                        rr0 = r0 + t_rows - L
                        # fp16 out-DMAs ride the gpsimd SWDGE path: narrow
                        # HWDGE fp16 transfers corrupted data earlier, and
                        # SWDGE also balances load across the DGE paths.
                        nc.gpsimd.dma_start(
                            out=rc_d[b, rr0:rr0 + (rows - t_rows), :],
                            in_=sr[t_rows:rows, :],
                        )

    nc.compile()
    return nc


def _numpy_fallback(x):
    """Reference math on host — only for unexpected shapes/degenerate L."""
    B, S, D = x.shape
    C = _dct_matrix64(S).astype(np.float32)
    x_dct = np.tensordot(C, x, axes=([1], [1])).transpose(1, 0, 2)  # (B,S,D)
    m = np.abs(x_dct.mean(axis=0).mean(axis=1))
    thr = np.quantile(m, 0.7)
    idx = np.nonzero(m > thr)[0]
    last_index = int(idx[-1]) if idx.size > 0 else -1
    trunc = x_dct[:, :last_index, :]
    L = trunc.shape[1]
    Cl = _dct_matrix64(L).astype(np.float32)
    recon = np.tensordot(Cl.T, trunc, axes=([1], [1])).transpose(1, 0, 2)
    return recon.astype(np.float16), np.ascontiguousarray(trunc)


def kernel(x, _trace=False):
    global LAST_RESULTS
    x = np.ascontiguousarray(np.asarray(x), dtype=np.float32)
    if x.shape != (_B, _S, _D):
        return _numpy_fallback(x)

    L = _resolve_L(x)
    if L < 1 or 2 * L < _P:
        return _numpy_fallback(x)

    Bc = _B // _NCORES
    key = (Bc, _S, _D, L)
    if key not in _CACHE:
        _CACHE[key] = _build_program(Bc, _S, _D, L)
    nc = _CACHE[key]

    _ensure_paths()
    if not _trace:
        # the NTFF trace path needs antenv.axon_hooks, absent on some
        # images; make sure a stray BASS_TRACE env can't send us there
        os.environ["BASS_NEVER_TRACE"] = "1"
    from concourse.bass_utils import run_bass_kernel_spmd

    wt = _build_weights(_S, L)
    x16 = x.astype(np.float16)
    in_maps = [{"x": x16[i * Bc:(i + 1) * Bc], "wt": wt} for i in range(_NCORES)]
    res = run_bass_kernel_spmd(nc, in_maps, list(range(_NCORES)), trace=_trace)
    LAST_RESULTS = res

    recon = np.concatenate([res.results[i]["recon"] for i in range(_NCORES)], axis=0)
    trunc = np.concatenate([res.results[i]["trunc"] for i in range(_NCORES)], axis=0)
    return recon.astype(np.float16, copy=False), trunc.astype(np.float32, copy=False)

